# revision 1
# baseline (speedup 1.0000x reference)
"""FFTConv2d kernel for trn2, 8 NeuronCores.

Math: reference einsum 'bchw,oihw->bohw' factorizes:
  Y[b,o] = conv_full(sum_c x[b,c], sum_i w[o,i])[1:-1,1:-1] + bias[o]
i.e. a single-channel 3x3 "same" convolution (flipped kernel) per (b,o).

Per core (2 batches):
  1. DMA x slice in as bf16 hi/lo pair (exact fp32 split), packed so each
     slice is one contiguous DMA; partitions=(b,c).
  2. Channel-sum via PE matmul with ones-indicator lhsT -> PSUM [6, n]
     (3 replicated copies per batch), accumulating hi+lo passes.
  3. Copy PSUM -> padded staging SBUF [6, 34*130] (row stride 130, zero
     borders), rounding to fp32r.
  4. Build P3 [8, 34*130]: partition (b,g) = staging col-shifted by (2-g);
     one contiguous SBUF->SBUF DMA each. Partitions (b,3) hold ones (bias).
  5. Conv: per 3-row output chunk, 3 accumulating fp32r matmuls (one per
     kernel row j) with rhs offset (2-j)*130 into P3 -> PSUM [128, 3, 130];
     all (b,o) images at once; bias rides the j=0 matmul's ones row.
  6. Copy PSUM -> Y SBUF (dropping the 2 pad columns per 130-row),
     DMA Y -> HBM.
Processed in NS row-slices for DMA/compute overlap.
"""

import os
import sys
from functools import lru_cache

import numpy as np

for _p in ("/opt/trn_rl_repo", "/root/.axon_site/_ro/trn_rl_repo"):
    if os.path.isdir(_p) and _p not in sys.path:
        sys.path.insert(0, _p)

import ml_dtypes

B, CIN, COUT, H, W = 16, 64, 64, 128, 128
N_CORES = 8
BPC = B // N_CORES  # batches per core = 2
NS = 4  # row slices per core
SH = H // NS  # rows per slice = 32
WROW = W + 2  # padded row stride = 130
PWIN = SH * WROW  # conv output window per slice = 4160
P3LEN = PWIN + 2 * WROW  # P3 length = 4420
SPLEN = P3LEN + 2  # staging length = 4422
NPART = BPC * CIN  # 128 input partitions (b, c)
NOUT = BPC * COUT  # 128 output partitions (b, o)
RMAX = SH + 2


def _slice_rows(s):
    h0 = max(0, SH * s - 1)
    he = min(H, SH * s + SH + 1)
    return h0, he


# packed input layout: per slice [hi rows | lo rows], contiguous
_SLICE_OFF = []
_off = 0
for _s in range(NS):
    _h0, _he = _slice_rows(_s)
    _SLICE_OFF.append(_off)
    _off += 2 * (_he - _h0) * W
XPACK_LEN = _off


@lru_cache(maxsize=1)
def _build():
    import concourse.bacc as bacc
    import concourse.mybir as mybir
    import concourse.tile as tile
    from concourse.ap import AP

    f32 = mybir.dt.float32
    f32r = mybir.dt.float32r
    bf16 = mybir.dt.bfloat16

    nc = bacc.Bacc("TRN2", target_bir_lowering=False, debug=False, num_devices=N_CORES)

    xp = nc.dram_tensor("xpack", [NPART, XPACK_LEN], bf16, kind="ExternalInput")
    ones_cs = nc.dram_tensor("ones_cs", [NPART, BPC * 3], bf16, kind="ExternalInput")
    wb = nc.dram_tensor("wb", [BPC * 9 + 1, NOUT], f32r, kind="ExternalInput")
    ones_p = nc.dram_tensor("ones_p", [1, PWIN], f32r, kind="ExternalInput")
    y = nc.dram_tensor("y", [NOUT, H * W], f32, kind="ExternalOutput")

    with tile.TileContext(nc) as tc:
        with (
            tc.tile_pool(name="xin", bufs=4) as xin_pool,
            tc.tile_pool(name="sp", bufs=1) as sp_pool,
            tc.tile_pool(name="pbuf", bufs=1) as p_pool,
            tc.tile_pool(name="yout", bufs=2) as y_pool,
            tc.tile_pool(name="consts", bufs=1) as c_pool,
            tc.tile_pool(name="cs_ps", bufs=4, space="PSUM") as cs_psum,
            tc.tile_pool(name="cv_ps", bufs=4, space="PSUM") as cv_psum,
        ):
            ones_t = c_pool.tile([NPART, BPC * 3], bf16, tag="ones_cs")
            nc.scalar.dma_start(out=ones_t[:, :], in_=ones_cs.ap()[:, :])
            wb_t = c_pool.tile([BPC * 9 + 1, NOUT], f32r, tag="wb")
            nc.scalar.dma_start(out=wb_t[:, :], in_=wb.ap()[:, :])

            # rotating staging + P3 + P9 buffers (zero borders persist)
            NBUF = 2
            NBUF9 = 3
            spbufs = []
            p9bufs = []
            for pi in range(NBUF):
                sp = sp_pool.tile([BPC * 3, SPLEN], f32r, tag=f"SP{pi}")
                spt0 = sp.tensor
                nc.vector.memset(
                    AP(tensor=spt0, offset=WROW - 1,
                       ap=[[SPLEN, BPC * 3], [WROW, RMAX], [1, 2]]).bitcast(f32),
                    0.0,
                )
                nc.vector.memset(sp[:, 0:WROW].bitcast(f32), 0.0)
                nc.vector.memset(sp[:, SPLEN - 1 : SPLEN].bitcast(f32), 0.0)
                spbufs.append(sp)
            for pi in range(NBUF9):
                p9 = p_pool.tile([BPC * 9 + 1, PWIN], f32r, tag=f"P9{pi}")
                nc.sync.dma_start(
                    out=p9[BPC * 9 : BPC * 9 + 1, :], in_=ones_p.ap()[0:1, :]
                )
                p9bufs.append(p9)

            def emit_in(s):
                h0, he = _slice_rows(s)
                ncols = (he - h0) * W
                xin = xin_pool.tile([NPART, 2 * RMAX * W], bf16, tag="xin")
                o = _SLICE_OFF[s]
                if s == 0:
                    # finer pieces so the first matmuls start sooner
                    for a0, a1 in ((0, 2048), (2048, ncols)):
                        nc.scalar.dma_start(
                            out=xin[:, a0:a1], in_=xp.ap()[:, o + a0 : o + a1]
                        )
                    for a0, a1 in ((0, 2048), (2048, ncols)):
                        nc.scalar.dma_start(
                            out=xin[:, ncols + a0 : ncols + a1],
                            in_=xp.ap()[:, o + ncols + a0 : o + ncols + a1],
                        )
                else:
                    nc.scalar.dma_start(
                        out=xin[:, :ncols], in_=xp.ap()[:, o : o + ncols]
                    )
                    nc.scalar.dma_start(
                        out=xin[:, ncols : 2 * ncols],
                        in_=xp.ap()[:, o + ncols : o + 2 * ncols],
                    )
                return xin

            def emit_cs_and_p(s, xin):
                hbase = SH * s - 1  # staging v-row 0 = image row hbase
                h0, he = _slice_rows(s)
                ncols = (he - h0) * W
                sp = spbufs[s % NBUF]
                spt = sp.tensor
                p9 = p9bufs[s % NBUF9]

                if s == NS - 1:
                    # bottom border: zero staging rows beyond image row 127
                    vz = (H - hbase) * WROW
                    nc.vector.memset(sp[:, vz:SPLEN].bitcast(f32), 0.0)

                # channel sum: ones^T @ [xhi; xlo], PSUM -> padded staging
                nchunks = (ncols + 511) // 512
                for ci in range(nchunks):
                    c0 = ci * 512
                    cn = min(512, ncols - c0)
                    nrows = cn // W
                    ps = cs_psum.tile([BPC * 3, 4, W], f32, tag="cs")
                    nc.tensor.matmul(
                        ps[:, :nrows, :],
                        ones_t[:, :],
                        xin[:, c0 : c0 + cn],
                        start=True,
                        stop=False,
                    )
                    nc.tensor.matmul(
                        ps[:, :nrows, :],
                        ones_t[:, :],
                        xin[:, ncols + c0 : ncols + c0 + cn],
                        start=False,
                        stop=True,
                    )
                    v0 = (h0 + 4 * ci - hbase) * WROW + 1
                    dst = AP(
                        tensor=spt,
                        offset=v0,
                        ap=[[SPLEN, BPC * 3], [WROW, nrows], [1, W]],
                    )
                    src = ps[:, :nrows, :]
                    if ci % 2 == 0:
                        nc.vector.tensor_copy(dst, src)
                    else:
                        nc.scalar.copy(dst, src)

                # build P9 single-hop: one DMA per (i,jj), both batches at
                # once (dst partitions 3i+jj and 9+3i+jj, stride 9).
                # P9[b*9+3i+jj, u] = sp[b*3+i, i... shifted]:
                #   = xp_b[32s*130 + u + jj*130 + (2-i)]
                spt_ = sp.tensor
                p9t = p9.tensor
                dmae = [nc.gpsimd, nc.gpsimd, nc.scalar]
                for i in range(3):
                    for jj in range(3):
                        m = 3 * i + jj
                        dmae[m % 3].dma_start(
                            out=AP(
                                tensor=p9t,
                                offset=m * PWIN,
                                ap=[[9 * PWIN, BPC], [1, PWIN]],
                            ),
                            in_=AP(
                                tensor=spt_,
                                offset=i * SPLEN + jj * WROW + 2 - i,
                                ap=[[3 * SPLEN, BPC], [1, PWIN]],
                            ),
                            single_packet=True,
                        )
                return p9

            def emit_warm():
                # dep-free matmuls that the PE chews on while waiting for a
                # P9 chain; keeps the HAM clock-gate at full rate.
                for _ in range(6):
                    ps = cs_psum.tile([BPC * 3, 4, W], f32, tag="cs")
                    nc.tensor.matmul(
                        ps[:, :, :],
                        ones_t[:, :],
                        xins[0][:, 0:512],
                        start=True,
                        stop=True,
                    )

            def emit_cv_and_out(s, p9):
                # conv: one K=20 fp32r matmul per 3-row chunk + psum->yt->hbm
                yt = y_pool.tile([NOUT, SH, W], f32, tag="yout")
                nchunk = (SH + 2) // 3
                for c in range(nchunk):
                    rr0 = c * 3
                    nrr = min(3, SH - rr0)
                    nn = nrr * WROW
                    ps = cv_psum.tile([NOUT, 3, WROW], f32, tag="cv")
                    nc.tensor.matmul(
                        ps[:, :nrr, :],
                        wb_t[:, :],
                        p9[:, rr0 * WROW : rr0 * WROW + nn],
                        start=True,
                        stop=True,
                    )
                    if c % 2 == 0:
                        nc.vector.tensor_copy(
                            yt[:, rr0 : rr0 + nrr, :], ps[:, :nrr, 0:W]
                        )
                    else:
                        nc.scalar.copy(yt[:, rr0 : rr0 + nrr, :], ps[:, :nrr, 0:W])

                half = SH // 2
                nc.sync.dma_start(
                    out=y.ap()[:, SH * s * W : (SH * s + half) * W],
                    in_=yt[:, :half, :],
                )
                nc.sync.dma_start(
                    out=y.ap()[:, (SH * s + half) * W : SH * (s + 1) * W],
                    in_=yt[:, half:, :],
                )

            # software-pipelined emission, two cs-stages ahead: PE stream is
            # cs0 cs1 cs2 cv0 cs3 cv1 cv2 cv3 so conv never heads the queue
            # while its P-build chain is still in flight.  Input DMAs are
            # emitted one slice ahead so they never queue behind P-chain
            # waits on their engine.
            DEPTH = 2
            p9s = {}
            xins = {s: emit_in(s) for s in range(NS)}
            for s in range(NS + DEPTH):
                if s < NS:
                    p9s[s] = emit_cs_and_p(s, xins[s])
                if s >= DEPTH:
                    emit_warm()
                    emit_cv_and_out(s - DEPTH, p9s[s - DEPTH])

    nc.compile()
    return nc


def _host_prep(x, weight, bias):
    bf = ml_dtypes.bfloat16
    wsum = weight.sum(axis=1)  # [COUT, 3, 3]
    wb = np.zeros((BPC * 9 + 1, NOUT), np.float32)
    for b in range(BPC):
        for i in range(3):
            for jj in range(3):
                wb[b * 9 + i * 3 + jj, b * COUT : (b + 1) * COUT] = wsum[
                    :, 2 - jj, i
                ]
    wb[BPC * 9, :] = np.tile(bias, BPC)
    ones_cs = np.zeros((NPART, BPC * 3), np.float32)
    for b in range(BPC):
        ones_cs[b * CIN : (b + 1) * CIN, b * 3 : (b + 1) * 3] = 1.0
    ones_cs = ones_cs.astype(bf)
    ones_p = np.ones((1, PWIN), np.float32)

    in_maps = []
    for r in range(N_CORES):
        xs = np.ascontiguousarray(
            x[r * BPC : (r + 1) * BPC].reshape(NPART, H, W)
        ).astype(np.float32)
        xhi = xs.astype(bf)
        xlo = (xs - xhi.astype(np.float32)).astype(bf)
        xpack = np.empty((NPART, XPACK_LEN), dtype=bf)
        for s in range(NS):
            h0, he = _slice_rows(s)
            n = (he - h0) * W
            o = _SLICE_OFF[s]
            xpack[:, o : o + n] = xhi[:, h0:he].reshape(NPART, n)
            xpack[:, o + n : o + 2 * n] = xlo[:, h0:he].reshape(NPART, n)
        in_maps.append(
            {
                "xpack": xpack,
                "ones_cs": ones_cs,
                "wb": wb,
                "ones_p": ones_p,
            }
        )
    return in_maps


def kernel(x, weight, bias):
    from concourse.bass_utils import run_bass_kernel_spmd

    x = np.asarray(x)
    weight = np.asarray(weight)
    bias = np.asarray(bias)
    nc = _build()
    in_maps = _host_prep(x, weight, bias)
    res = run_bass_kernel_spmd(nc, in_maps, core_ids=list(range(N_CORES)))
    out = np.concatenate(
        [
            res.results[r]["y"].reshape(BPC, COUT, H, W)
            for r in range(N_CORES)
        ],
        axis=0,
    )
    return out.astype(np.float32)



# revision 8
# speedup vs baseline: 1.0778x; 1.0778x over previous
"""FFTConv2d kernel for trn2, 8 NeuronCores.

Math: reference einsum 'bchw,oihw->bohw' factorizes:
  Y[b,o] = conv_same(sum_c x[b,c], flip(sum_i w[o,i])) + bias[o]
i.e. a single-channel 3x3 "same" convolution per (b,o) pair.

Per core (2 batches), all SBUF data in fp16 (PSUM accum fp32):
  1. xin [128,(b,c), 16384] <- x as fp16, 9 HBM DMA pieces (SP ring).
  2. Channel-sum via PE matmul with ones indicator [128,2] -> PSUM
     [2, 2x4x128] (paired 4-row chunks), 32 chunks total.
  3. PSUM -> sp [2, 16902] fp16 padded staging (row stride 130, zero
     borders memset once); one paired copy per 8 rows (DVE/Act alt).
  4. P9 [19, 16640]: 9 shifted whole-image views, partition m=3i'+jj
     holds sp shifted by jj*130+i' -- so dst strides are partition
     aligned and each 32-row band is ONE regular gpsimd DMA (4 total).
     Partition 18 = ones (bias rides the conv matmul).
  5. Conv: per 3-row chunk a single K=19 fp16 matmul (N=390) -> PSUM;
     paired chunks share a 2-bank PSUM tile; paired copy -> yt fp16.
  6. yt -> HBM in 8 pieces (SP ring), y fp16 upconverted on host.
"""

import os
import sys
from functools import lru_cache

import numpy as np

for _p in ("/opt/trn_rl_repo", "/root/.axon_site/_ro/trn_rl_repo"):
    if os.path.isdir(_p) and _p not in sys.path:
        sys.path.insert(0, _p)

B, CIN, COUT, H, W = 16, 64, 64, 128, 128
N_CORES = 8
BPC = B // N_CORES  # 2
NPART = BPC * CIN  # 128 input partitions (b, c)
NOUT = BPC * COUT  # 128 output partitions (b, o)
WROW = W + 2  # 130
HW = H * W  # 16384
P9LEN = H * WROW  # 16640
LSP = (H + 2) * WROW + 2  # 16902
NK = BPC * 9 + 1  # 19


@lru_cache(maxsize=1)
def _build():
    import concourse.bacc as bacc
    import concourse.mybir as mybir
    import concourse.tile as tile
    from concourse.ap import AP

    f32 = mybir.dt.float32
    f16 = mybir.dt.float16

    nc = bacc.Bacc("TRN2", target_bir_lowering=False, debug=False, num_devices=N_CORES)

    xh = nc.dram_tensor("xh", [NPART, HW], f16, kind="ExternalInput")
    wbh = nc.dram_tensor("wb", [NK, NOUT], f16, kind="ExternalInput")
    onesp = nc.dram_tensor("ones_p", [1, P9LEN], f16, kind="ExternalInput")
    y = nc.dram_tensor("y", [NOUT, HW], f16, kind="ExternalOutput")
    dump = os.environ.get("KDUMP")
    if dump:
        sp_d = nc.dram_tensor("sp_d", [BPC, LSP], f16, kind="ExternalOutput")
        p9_d = nc.dram_tensor("p9_d", [NK, P9LEN], f16, kind="ExternalOutput")

    with tile.TileContext(nc) as tc:
        with (
            tc.tile_pool(name="main", bufs=1) as mp,
            tc.tile_pool(name="cs_ps", bufs=1, space="PSUM") as cs_ps,
            tc.tile_pool(name="cv_ps", bufs=1, space="PSUM") as cv_ps,
        ):
            xin = mp.tile([NPART, HW], f16, tag="xin")
            sp = mp.tile([BPC, LSP], f16, tag="sp")
            p9 = mp.tile([NK, P9LEN], f16, tag="p9")
            yt = mp.tile([NOUT, HW], f16, tag="yt")
            ones_t = mp.tile([NPART, BPC], f16, tag="ones")
            wb_t = mp.tile([NK, NOUT], f16, tag="wb")

            spt = sp.tensor
            p9t = p9.tensor
            ytt = yt.tensor

            # rotating 2-bank PSUM tiles (manual tags, 8 banks total)
            csb = [
                cs_ps.tile([BPC, 2, 4, W], f32, tag=f"cs{i}", name=f"cs{i}")
                for i in range(2)
            ]
            cvb = [
                cv_ps.tile([NOUT, 2, 512], f32, tag=f"cv{i}", name=f"cv{i}")
                for i in range(2)
            ]

            # constants (Act HWDGE ring)
            nc.scalar.dma_start(out=wb_t[:, :], in_=wbh.ap()[:, :])
            nc.scalar.dma_start(out=p9[NK - 1 : NK, :], in_=onesp.ap()[0:1, :])

            # ones indicator [128, 2]: col b is 1 for partitions of batch b
            nc.vector.memset(ones_t[0:CIN, 0:1], 1.0)
            nc.vector.memset(ones_t[0:CIN, 1:2], 0.0)
            nc.vector.memset(ones_t[CIN:NPART, 0:1], 0.0)
            nc.vector.memset(ones_t[CIN:NPART, 1:2], 1.0)

            # sp zero borders: row -1, row 128 + tail, and col pairs
            # (right col of row h, left col of row h+1)
            nc.vector.memset(sp[:, 0:WROW], 0.0)
            nc.vector.memset(sp[:, (H + 1) * WROW : LSP], 0.0)
            nc.vector.memset(
                AP(tensor=spt, offset=WROW - 1, ap=[[LSP, BPC], [WROW, H + 1], [1, 2]]),
                0.0,
            )

            # input: 9 pieces on the SP ring
            pieces = [(0, 1024), (1024, 1024)] + [
                (2048 * q, 2048) for q in range(1, 8)
            ]
            for o, n in pieces:
                nc.sync.dma_start(out=xin[:, o : o + n], in_=xh.ap()[:, o : o + n])

            copy_engines = [nc.vector, nc.scalar]

            def ecopy(idx, dst, src):
                eng = copy_engines[idx % 2]
                if eng is nc.vector:
                    eng.tensor_copy(dst, src)
                else:
                    eng.copy(dst, src)

            def emit_cs_pair(kp):
                ps = csb[kp % 2]
                for h in range(2):
                    c0 = 1024 * kp + 512 * h
                    nc.tensor.matmul(
                        ps[:, h, :, :],
                        ones_t[:, :],
                        xin[:, c0 : c0 + 512],
                        start=True,
                        stop=True,
                    )
                dst = AP(
                    tensor=spt,
                    offset=(8 * kp + 1) * WROW + 1,
                    ap=[[LSP, BPC], [4 * WROW, 2], [WROW, 4], [1, W]],
                )
                ecopy(kp, dst, ps[:, :, :, :])

            def emit_band(r0, r1, engines):
                # P9 rows [r0, r1): one 2-D DMA per (i', jj) shift copy
                ln = (r1 - r0) * WROW
                n = 0
                for ip in range(3):
                    for jj in range(3):
                        eng = engines[n % len(engines)]
                        n += 1
                        eng.dma_start(
                            out=AP(
                                tensor=p9t,
                                offset=(3 * ip + jj) * P9LEN + r0 * WROW,
                                ap=[[9 * P9LEN, BPC], [1, ln]],
                            ),
                            in_=AP(
                                tensor=spt,
                                offset=(r0 + jj) * WROW + ip,
                                ap=[[LSP, BPC], [1, ln]],
                            ),
                            single_packet=(eng is nc.gpsimd),
                        )

            def emit_conv_pair(j0):
                cv = cvb[(j0 // 2) % 2]
                cvt = cv.tensor
                nj = 2 if j0 + 1 < 43 else 1
                for h in range(nj):
                    j = j0 + h
                    nr = min(3, H - 3 * j)
                    nn = nr * WROW
                    nc.tensor.matmul(
                        AP(tensor=cvt, offset=512 * h, ap=[[1024, NOUT], [1, nn]]),
                        wb_t[:, :],
                        p9[:, 390 * j : 390 * j + nn],
                        start=True,
                        stop=True,
                    )
                if nj == 2:
                    src = AP(
                        tensor=cvt,
                        offset=0,
                        ap=[[1024, NOUT], [512, 2], [WROW, 3], [1, W]],
                    )
                    dst = AP(
                        tensor=ytt,
                        offset=384 * j0,
                        ap=[[HW, NOUT], [384, 2], [W, 3], [1, W]],
                    )
                else:  # tail: j=42, 2 rows
                    src = AP(
                        tensor=cvt, offset=0, ap=[[1024, NOUT], [WROW, 2], [1, W]]
                    )
                    dst = AP(
                        tensor=ytt, offset=384 * j0, ap=[[HW, NOUT], [W, 2], [1, W]]
                    )
                ecopy(j0 // 2, dst, src)

            def emit_out(q):
                nc.sync.dma_start(
                    out=y.ap()[:, 2048 * q : 2048 * (q + 1)],
                    in_=yt[:, 2048 * q : 2048 * (q + 1)],
                )

            # piece q of output is ready after the conv pair covering row 16q+15
            out_after = {4: 0, 10: 1, 14: 2, 20: 3, 26: 4, 30: 5, 36: 6, 42: 7}

            for kp in range(16):
                emit_cs_pair(kp)
                if kp == 6:
                    # band A (P9 rows 0..47) needs sp rows <= 48 (cs pair 6)
                    emit_band(0, 48, [nc.gpsimd])
            # band B (rows 48..127) needs all cs; split Pool/SP for latency
            emit_band(48, H, [nc.gpsimd, nc.gpsimd, nc.sync])
            for j0 in list(range(0, 42, 2)) + [42]:
                emit_conv_pair(j0)
                if j0 in out_after:
                    emit_out(out_after[j0])
            if dump:
                nc.sync.dma_start(out=sp_d.ap()[:, :], in_=sp[:, :])
                nc.sync.dma_start(out=p9_d.ap()[:, :], in_=p9[:, :])

    nc.compile()
    return nc


def _host_prep(x, weight, bias):
    wsum = weight.sum(axis=1)  # [COUT, 3, 3]
    wb = np.zeros((NK, NOUT), np.float32)
    for b in range(BPC):
        for ip in range(3):
            for jj in range(3):
                wb[b * 9 + 3 * ip + jj, b * COUT : (b + 1) * COUT] = wsum[
                    :, 2 - jj, 2 - ip
                ]
    wb[NK - 1, :] = np.tile(bias, BPC)
    wb = wb.astype(np.float16)
    ones_p = np.ones((1, P9LEN), np.float16)

    in_maps = []
    for r in range(N_CORES):
        xhr = np.ascontiguousarray(
            x[r * BPC : (r + 1) * BPC].reshape(NPART, HW)
        ).astype(np.float16)
        in_maps.append({"xh": xhr, "wb": wb, "ones_p": ones_p})
    return in_maps


def kernel(x, weight, bias):
    from concourse.bass_utils import run_bass_kernel_spmd

    x = np.asarray(x)
    weight = np.asarray(weight)
    bias = np.asarray(bias)
    nc = _build()
    in_maps = _host_prep(x, weight, bias)
    res = run_bass_kernel_spmd(nc, in_maps, core_ids=list(range(N_CORES)))
    out = np.concatenate(
        [
            np.asarray(res.results[r]["y"])
            .astype(np.float32)
            .reshape(BPC, COUT, H, W)
            for r in range(N_CORES)
        ],
        axis=0,
    )
    return out


# revision 11
# speedup vs baseline: 1.2578x; 1.1670x over previous
"""FFTConv2d kernel for trn2, 8 NeuronCores.

Math: reference einsum 'bchw,oihw->bohw' factorizes:
  Y[b,o] = conv_same(sum_c x[b,c], flip(sum_i w[o,i])) + bias[o]
i.e. a single-channel 3x3 "same" convolution per (b,o) pair.

Per core (2 batches), all SBUF data fp16 (PSUM accum fp32):
  1. xin [128 (b,c), 16384] <- x fp16, 9 HBM DMA pieces (SP ring).
  2. Channel-sum: ones-indicator matmul pairs -> PSUM [2, 2x512];
     one FD=1024 copy (DVE/Act alternating) per pair drains 8 image
     rows into the padded staging = P9 partitions {0,1} (row stride
     130, zero borders memset once).
  3. P9 [19, 16902], partition 2m+b holds staging shifted by
     jj*130+i' (m=3i'+jj); m=0 IS the staging; m=1..8 built by 2-D
     self-copy DMAs in 2 row-bands (8 DMAs each, Pool/SP).
     Partition 18 = ones (bias rides the conv matmul).
  4. Conv: 33 flat 512-col chunks (yt rows are 130 wide, 2 junk cols
     stripped on host); K=19 fp16 matmul pairs into the same 4-deep
     2-bank PSUM rotation; FD=1024 contiguous copies -> yt.
  5. yt [128, 16640] -> HBM in 8 pieces on the Pool (SWDGE) ring,
     fp16; host strips junk cols + upconverts to fp32.

PSUM: four [128, 1024] fp32 tensors (2 banks each = all 8 banks),
time-shared: cs pairs use partitions 0:2, conv pairs all 128.
"""

import os
import sys
from functools import lru_cache

import numpy as np

for _p in ("/opt/trn_rl_repo", "/root/.axon_site/_ro/trn_rl_repo"):
    if os.path.isdir(_p) and _p not in sys.path:
        sys.path.insert(0, _p)

B, CIN, COUT, H, W = 16, 64, 64, 128, 128
N_CORES = 8
BPC = B // N_CORES  # 2
NPART = BPC * CIN  # 128
NOUT = BPC * COUT  # 128
WROW = W + 2  # 130
HW = H * W  # 16384
HHW = H * WROW  # 16640 (130-wide output rows)
LSP = (H + 2) * WROW + 2  # 16902 (padded staging length)
NK = BPC * 9 + 1  # 19
NCV = 33  # conv chunks: 32x512 + 1x256


@lru_cache(maxsize=1)
def _build():
    import concourse.bacc as bacc
    import concourse.mybir as mybir
    import concourse.tile as tile
    from concourse.ap import AP

    f32 = mybir.dt.float32
    f16 = mybir.dt.float16

    nc = bacc.Bacc("TRN2", target_bir_lowering=False, debug=False, num_devices=N_CORES)

    xh = nc.dram_tensor("xh", [NPART, HW], f16, kind="ExternalInput")
    wbh = nc.dram_tensor("wb", [NK, NOUT], f16, kind="ExternalInput")
    onesp = nc.dram_tensor("ones_p", [1, HHW], f16, kind="ExternalInput")
    y = nc.dram_tensor("y", [NOUT, HHW], f16, kind="ExternalOutput")
    dump = os.environ.get("KDUMP")
    if dump:
        p9_d = nc.dram_tensor("p9_d", [NK, HHW], f16, kind="ExternalOutput")

    with tile.TileContext(nc) as tc:
        with (
            tc.tile_pool(name="main", bufs=1) as mp,
            tc.tile_pool(name="ps", bufs=1, space="PSUM") as ps_pool,
        ):
            xin = mp.tile([NPART, HW], f16, tag="xin")
            p9 = mp.tile([NK, LSP], f16, tag="p9")
            yt = mp.tile([NOUT, HHW], f16, tag="yt")
            ones_t = mp.tile([NPART, BPC], f16, tag="ones_t")
            wb_t = mp.tile([NK, NOUT], f16, tag="wb")

            p9t = p9.tensor

            psb = [
                ps_pool.tile([NOUT, 1024], f32, tag=f"ps{i}", name=f"ps{i}")
                for i in range(4)
            ]

            # constants (Act HWDGE ring)
            nc.scalar.dma_start(out=wb_t[:, :], in_=wbh.ap()[:, :])
            nc.scalar.dma_start(out=p9[NK - 1 : NK, 0:HHW], in_=onesp.ap()[0:1, :])

            # ones indicator [128, 2]: col b is 1 for partitions of batch b
            nc.vector.memset(ones_t[0:CIN, 0:1], 1.0)
            nc.vector.memset(ones_t[0:CIN, 1:2], 0.0)
            nc.vector.memset(ones_t[CIN:NPART, 0:1], 0.0)
            nc.vector.memset(ones_t[CIN:NPART, 1:2], 1.0)

            # staging zero borders in P9 partitions {0, 1}:
            # row -1, row 128 + tail, and (right col, next left col) pairs
            nc.vector.memset(
                AP(tensor=p9t, offset=0, ap=[[LSP, BPC], [1, WROW]]), 0.0
            )
            nc.vector.memset(
                AP(
                    tensor=p9t,
                    offset=(H + 1) * WROW,
                    ap=[[LSP, BPC], [1, LSP - (H + 1) * WROW]],
                ),
                0.0,
            )
            nc.vector.memset(
                AP(
                    tensor=p9t,
                    offset=WROW - 1,
                    ap=[[LSP, BPC], [WROW, H + 1], [1, 2]],
                ),
                0.0,
            )

            # input: 9 pieces on the SP ring
            pieces = [(0, 1024), (1024, 1024)] + [
                (2048 * q, 2048) for q in range(1, 8)
            ]
            for o, n in pieces:
                nc.sync.dma_start(out=xin[:, o : o + n], in_=xh.ap()[:, o : o + n])

            copy_engines = [nc.vector, nc.scalar]

            def ecopy(idx, dst, src):
                eng = copy_engines[idx % 2]
                if eng is nc.vector:
                    eng.tensor_copy(dst, src)
                else:
                    eng.copy(dst, src)

            def emit_cs_pair(kp):
                # 2 ones-matmuls -> PSUM [2, 2, 512]; 1 copy -> 8 rows
                ps = psb[kp % 4]
                pst = ps.tensor
                for h in range(2):
                    c0 = 1024 * kp + 512 * h
                    nc.tensor.matmul(
                        ps[0:BPC, 512 * h : 512 * h + 512],
                        ones_t[:, :],
                        xin[:, c0 : c0 + 512],
                        start=True,
                        stop=True,
                    )
                dst = AP(
                    tensor=p9t,
                    offset=(8 * kp + 1) * WROW + 1,
                    ap=[[LSP, BPC], [4 * WROW, 2], [WROW, 4], [1, W]],
                )
                src = AP(
                    tensor=pst,
                    offset=0,
                    ap=[[1024, BPC], [512, 2], [W, 4], [1, W]],
                )
                ecopy(kp, dst, src)

            def emit_band(r0, r1, engines):
                # P9 partitions m=1..8 <- shifted copies of partitions {0,1}
                ln = (r1 - r0) * WROW
                n = 0
                for m in range(1, 9):
                    ip, jj = divmod(m, 3)
                    eng = engines[n % len(engines)]
                    n += 1
                    eng.dma_start(
                        out=AP(
                            tensor=p9t,
                            offset=2 * m * LSP + r0 * WROW,
                            ap=[[LSP, BPC], [1, ln]],
                        ),
                        in_=AP(
                            tensor=p9t,
                            offset=(r0 + jj) * WROW + ip,
                            ap=[[LSP, BPC], [1, ln]],
                        ),
                        single_packet=(eng is nc.gpsimd),
                    )

            def emit_conv_pair(jp):
                # 2 conv chunks (or 1 tail chunk) -> 1 contiguous copy
                cv = psb[(16 + jp) % 4]
                j0 = 2 * jp
                nj = 2 if j0 + 1 < NCV else 1
                tot = 0
                for h in range(nj):
                    j = j0 + h
                    nn = 512 if j < NCV - 1 else 256
                    nc.tensor.matmul(
                        cv[:, 512 * h : 512 * h + nn],
                        wb_t[:, :],
                        p9[:, 512 * j : 512 * j + nn],
                        start=True,
                        stop=True,
                    )
                    tot += nn
                ecopy(jp, yt[:, 512 * j0 : 512 * j0 + tot], cv[:, 0:tot])

            def emit_out(q):
                nc.gpsimd.dma_start(
                    out=y.ap()[:, 2080 * q : 2080 * (q + 1)],
                    in_=yt[:, 2080 * q : 2080 * (q + 1)],
                )

            # out piece q (rows 16q..) ready after conv pair: {pair: piece}
            out_after = {2: 0, 4: 1, 6: 2, 8: 3, 10: 4, 12: 5, 14: 6, 16: 7}

            for kp in range(16):
                emit_cs_pair(kp)
                if kp == 6:
                    # band A (P9 flat [0, 6240)) needs staging rows <= 48
                    emit_band(0, 48, [nc.gpsimd])
            emit_band(48, H, [nc.gpsimd, nc.gpsimd, nc.sync])
            for jp in range(17):
                emit_conv_pair(jp)
                if jp in out_after:
                    emit_out(out_after[jp])
            if dump:
                nc.sync.dma_start(out=p9_d.ap()[:, :], in_=p9[:, 0:HHW])

    nc.compile()
    return nc


def _host_prep(x, weight, bias):
    wsum = weight.sum(axis=1)  # [COUT, 3, 3]
    wb = np.zeros((NK, NOUT), np.float32)
    for b in range(BPC):
        for ip in range(3):
            for jj in range(3):
                wb[2 * (3 * ip + jj) + b, b * COUT : (b + 1) * COUT] = wsum[
                    :, 2 - jj, 2 - ip
                ]
    wb[NK - 1, :] = np.tile(bias, BPC)
    wb = wb.astype(np.float16)
    ones_p = np.ones((1, HHW), np.float16)

    in_maps = []
    for r in range(N_CORES):
        xhr = np.ascontiguousarray(
            x[r * BPC : (r + 1) * BPC].reshape(NPART, HW)
        ).astype(np.float16)
        in_maps.append({"xh": xhr, "wb": wb, "ones_p": ones_p})
    return in_maps


def kernel(x, weight, bias):
    from concourse.bass_utils import run_bass_kernel_spmd

    x = np.asarray(x)
    weight = np.asarray(weight)
    bias = np.asarray(bias)
    nc = _build()
    in_maps = _host_prep(x, weight, bias)
    res = run_bass_kernel_spmd(nc, in_maps, core_ids=list(range(N_CORES)))
    out = np.concatenate(
        [
            np.asarray(res.results[r]["y"])
            .astype(np.float32)
            .reshape(BPC, COUT, H, WROW)[:, :, :, :W]
            for r in range(N_CORES)
        ],
        axis=0,
    )
    return out


# revision 12
# speedup vs baseline: 1.4477x; 1.1510x over previous
"""FFTConv2d kernel for trn2, 8 NeuronCores.

Math: reference einsum 'bchw,oihw->bohw' factorizes:
  Y[b,o] = conv_same(sum_c x[b,c], flip(sum_i w[o,i])) + bias[o]
i.e. a single-channel 3x3 "same" convolution per (b,o) pair.

Per core (2 batches), all SBUF data fp16 (PSUM accum fp32):
  1. xin [128 (b,c), 16384] <- x fp16, 9 HBM DMA pieces (SP ring).
  2. Channel-sum: ones-indicator matmul pairs -> PSUM [2, 2x512];
     one FD=1024 copy (DVE/Act alternating) per pair drains 8 image
     rows into the padded staging = P9 partitions {0,1} (row stride
     130, zero borders memset once).
  3. P9 [19, 16902], partition 2m+b holds staging shifted by
     jj*130+i' (m=3i'+jj); m=0 IS the staging; m=1..8 built by 2-D
     self-copy DMAs in 2 row-bands (8 DMAs each, Pool/SP).
     Partition 18 = ones (bias rides the conv matmul).
  4. Conv: 33 flat 512-col chunks (yt rows are 130 wide, 2 junk cols
     stripped on host); K=19 fp16 matmul pairs into the same 4-deep
     2-bank PSUM rotation; FD=1024 contiguous copies -> yt.
  5. yt [128, 16640] -> HBM in 8 pieces on the Pool (SWDGE) ring,
     fp16; host strips junk cols + upconverts to fp32.

PSUM: four [128, 1024] fp32 tensors (2 banks each = all 8 banks),
time-shared: cs pairs use partitions 0:2, conv pairs all 128.
"""

import os
import sys
from functools import lru_cache

import numpy as np

for _p in ("/opt/trn_rl_repo", "/root/.axon_site/_ro/trn_rl_repo"):
    if os.path.isdir(_p) and _p not in sys.path:
        sys.path.insert(0, _p)

B, CIN, COUT, H, W = 16, 64, 64, 128, 128
N_CORES = 8
BPC = B // N_CORES  # 2
NPART = BPC * CIN  # 128
NOUT = BPC * COUT  # 128
WROW = W + 2  # 130
HW = H * W  # 16384
HHW = H * WROW  # 16640 (130-wide output rows)
LSP = (H + 2) * WROW + 2  # 16902 (padded staging length)
NK = BPC * 9 + 1  # 19
NCV = 33  # conv chunks: 32x512 + 1x256


@lru_cache(maxsize=1)
def _build():
    import concourse.bacc as bacc
    import concourse.mybir as mybir
    import concourse.tile as tile
    from concourse.ap import AP

    f32 = mybir.dt.float32
    f16 = mybir.dt.float16

    nc = bacc.Bacc("TRN2", target_bir_lowering=False, debug=False, num_devices=N_CORES)

    xh = nc.dram_tensor("xh", [NPART, HW], f16, kind="ExternalInput")
    wbh = nc.dram_tensor("wb", [NK, NOUT], f16, kind="ExternalInput")
    onesp = nc.dram_tensor("ones_p", [1, HHW], f16, kind="ExternalInput")
    y = nc.dram_tensor("y", [NOUT, HHW], f16, kind="ExternalOutput")
    dump = os.environ.get("KDUMP")
    if dump:
        p9_d = nc.dram_tensor("p9_d", [NK, HHW], f16, kind="ExternalOutput")

    with tile.TileContext(nc) as tc:
        with (
            tc.tile_pool(name="main", bufs=1) as mp,
            tc.tile_pool(name="ps", bufs=1, space="PSUM") as ps_pool,
        ):
            xin = mp.tile([NPART, HW], f16, tag="xin")
            p9 = mp.tile([NK, LSP], f16, tag="p9")
            yt = mp.tile([NOUT, HHW], f16, tag="yt")
            ones_t = mp.tile([NPART, BPC], f16, tag="ones_t")
            wb_t = mp.tile([NK, NOUT], f16, tag="wb")

            p9t = p9.tensor

            csb = [
                ps_pool.tile([BPC, 512], f32, tag=f"cs{i}", name=f"cs{i}")
                for i in range(4)
            ]
            cvb = [
                ps_pool.tile([NOUT, 512], f32, tag=f"cv{i}", name=f"cv{i}")
                for i in range(4)
            ]

            # constants on the idle Pool (SWDGE) ring so the Act queue
            # starts clean (a late Act start skews the static schedule)
            nc.gpsimd.dma_start(out=wb_t[:, :], in_=wbh.ap()[:, :])
            nc.gpsimd.dma_start(out=p9[NK - 1 : NK, 0:HHW], in_=onesp.ap()[0:1, :])

            # ones indicator [128, 2]: col b is 1 for partitions of batch b
            nc.vector.memset(ones_t[0:CIN, 0:1], 1.0)
            nc.vector.memset(ones_t[0:CIN, 1:2], 0.0)
            nc.vector.memset(ones_t[CIN:NPART, 0:1], 0.0)
            nc.vector.memset(ones_t[CIN:NPART, 1:2], 1.0)

            # staging zero borders in P9 partitions {0, 1}:
            # row -1, row 128 + tail, and (right col, next left col) pairs
            nc.vector.memset(
                AP(tensor=p9t, offset=0, ap=[[LSP, BPC], [1, WROW]]), 0.0
            )
            nc.vector.memset(
                AP(
                    tensor=p9t,
                    offset=(H + 1) * WROW,
                    ap=[[LSP, BPC], [1, LSP - (H + 1) * WROW]],
                ),
                0.0,
            )
            nc.vector.memset(
                AP(
                    tensor=p9t,
                    offset=WROW - 1,
                    ap=[[LSP, BPC], [WROW, H + 1], [1, 2]],
                ),
                0.0,
            )

            # input: 9 pieces on the SP ring
            pieces = [(0, 1024), (1024, 1024)] + [
                (2048 * q, 2048) for q in range(1, 8)
            ]
            for o, n in pieces:
                nc.sync.dma_start(out=xin[:, o : o + n], in_=xh.ap()[:, o : o + n])

            copy_engines = [nc.vector, nc.scalar]

            def ecopy(idx, dst, src):
                eng = copy_engines[idx % 2]
                if eng is nc.vector:
                    eng.tensor_copy(dst, src)
                else:
                    eng.copy(dst, src)

            def emit_cs(k):
                # ones-matmul of 512 cols (4 rows) -> [2, 512]; 1 copy
                ps = csb[k % 4]
                pst = ps.tensor
                nc.tensor.matmul(
                    ps[:, :],
                    ones_t[:, :],
                    xin[:, 512 * k : 512 * k + 512],
                    start=True,
                    stop=True,
                )
                dst = AP(
                    tensor=p9t,
                    offset=(4 * k + 1) * WROW + 1,
                    ap=[[LSP, BPC], [WROW, 4], [1, W]],
                )
                src = AP(
                    tensor=pst, offset=0, ap=[[512, BPC], [W, 4], [1, W]]
                )
                ecopy(k, dst, src)

            def emit_band(r0, r1, engines):
                # P9 partitions m=1..8 <- shifted copies of partitions {0,1}
                ln = (r1 - r0) * WROW
                n = 0
                for m in range(1, 9):
                    ip, jj = divmod(m, 3)
                    eng = engines[n % len(engines)]
                    n += 1
                    eng.dma_start(
                        out=AP(
                            tensor=p9t,
                            offset=2 * m * LSP + r0 * WROW,
                            ap=[[LSP, BPC], [1, ln]],
                        ),
                        in_=AP(
                            tensor=p9t,
                            offset=(r0 + jj) * WROW + ip,
                            ap=[[LSP, BPC], [1, ln]],
                        ),
                        single_packet=(eng is nc.gpsimd),
                    )

            def emit_conv(j):
                cv = cvb[j % 4]
                nn = 512 if j < NCV - 1 else 256
                nc.tensor.matmul(
                    cv[:, :nn],
                    wb_t[:, :],
                    p9[:, 512 * j : 512 * j + nn],
                    start=True,
                    stop=True,
                )
                ecopy(j, yt[:, 512 * j : 512 * j + nn], cv[:, :nn])

            def emit_out(q):
                nc.gpsimd.dma_start(
                    out=y.ap()[:, 2080 * q : 2080 * (q + 1)],
                    in_=yt[:, 2080 * q : 2080 * (q + 1)],
                )

            # out piece q (rows 16q..) ready after conv chunk: {chunk: piece}
            out_after = {4: 0, 8: 1, 12: 2, 16: 3, 20: 4, 24: 5, 28: 6, 32: 7}

            for k in range(32):
                emit_cs(k)
                if k == 12:
                    # band A (P9 flat [0, 6240)) needs staging rows <= 48
                    emit_band(0, 48, [nc.gpsimd])
            emit_band(48, H, [nc.gpsimd, nc.gpsimd, nc.sync])
            for j in range(NCV):
                emit_conv(j)
                if j in out_after:
                    emit_out(out_after[j])
            if dump:
                nc.sync.dma_start(out=p9_d.ap()[:, :], in_=p9[:, 0:HHW])

    nc.compile()
    return nc


def _host_prep(x, weight, bias):
    wsum = weight.sum(axis=1)  # [COUT, 3, 3]
    wb = np.zeros((NK, NOUT), np.float32)
    for b in range(BPC):
        for ip in range(3):
            for jj in range(3):
                wb[2 * (3 * ip + jj) + b, b * COUT : (b + 1) * COUT] = wsum[
                    :, 2 - jj, 2 - ip
                ]
    wb[NK - 1, :] = np.tile(bias, BPC)
    wb = wb.astype(np.float16)
    ones_p = np.ones((1, HHW), np.float16)

    in_maps = []
    for r in range(N_CORES):
        xhr = np.ascontiguousarray(
            x[r * BPC : (r + 1) * BPC].reshape(NPART, HW)
        ).astype(np.float16)
        in_maps.append({"xh": xhr, "wb": wb, "ones_p": ones_p})
    return in_maps


def kernel(x, weight, bias):
    from concourse.bass_utils import run_bass_kernel_spmd

    x = np.asarray(x)
    weight = np.asarray(weight)
    bias = np.asarray(bias)
    nc = _build()
    in_maps = _host_prep(x, weight, bias)
    res = run_bass_kernel_spmd(nc, in_maps, core_ids=list(range(N_CORES)))
    out = np.concatenate(
        [
            np.asarray(res.results[r]["y"])
            .astype(np.float32)
            .reshape(BPC, COUT, H, WROW)[:, :, :, :W]
            for r in range(N_CORES)
        ],
        axis=0,
    )
    return out
